# revision 52
# baseline (speedup 1.0000x reference)
"""CHESHIRE hyperedge link predictor on 8 Trainium2 NeuronCores.

Structure exploited (verified at runtime):
  - members[e] = base[e] + arange(8): each hyperedge is a contiguous
    8-node window -> sorting hyperedges by base makes the whole problem
    embarrassingly parallel across a node-range partition (no collectives).
  - edge_index is the full directed 8-clique per hyperedge -> deg == 7,
    w == -1/7, so Lhat(h) = (h - sum(h))/7 and the K=3 ChebConv folds into
    a single per-entry GEMM plus a per-window GEMM:
        u_i = (r * x_i) @ Wap + (r * S) @ Wd + D0
    with r = 1/sqrt(var+eps) (GraphNorm fold), S = window sum of x.
  - clip commutes with max/min pools; clip(u)^2 == min(u^2, 1) for the rms
    pool, so hardtanh is applied only to pooled quantities.

Per-core device pipeline:
  encoder GEMM (f32r) -> +bias, hardtanh -> x to DRAM (bf16) -> dma_gather
  (transpose mode) produces xeT[ch, entry, window] directly -> window
  stats via free-axis reduces -> r, p=r*S -> per-entry GEMM with the
  per-window term accumulated into the same PSUM -> pools via free-axis
  reduces -> logits GEMM -> sigmoid -> out.
"""

import math

import numpy as np

N_CORES = 8
M = 8          # nodes per hyperedge
D = 128        # embedding dim
F = 512        # input feature dim
EPS = 1e-5
GSZ = 128      # windows per PSUM group

_GRAPH_CACHE = {}
LAST_EXEC_NS = None
LAST_RESULT = None


def _bf16_dtype():
    import ml_dtypes

    return np.dtype(ml_dtypes.bfloat16)


def _fold_weights(W_enc, b_enc, gn_gamma, gn_beta, gn_alpha, cheb_W, cheb_b,
                  lin_W, lin_b):
    f32 = np.float32
    W0, W1, W2 = (np.asarray(cheb_W[i], f32) for i in range(3))
    gam = np.asarray(gn_gamma, f32)
    bet = np.asarray(gn_beta, f32)
    alp = np.asarray(gn_alpha, f32)
    Wa = W0 + W1 / f32(7.0) - f32(47.0 / 49.0) * W2
    Wb = -W1 / f32(7.0) + f32(12.0 / 49.0) * W2
    Wap = gam[:, None] * Wa
    Wd = -(gam * alp / f32(8.0))[:, None] * (Wa + f32(8.0) * Wb) + gam[:, None] * Wb
    D0 = bet @ Wa + f32(8.0) * (bet @ Wb) + np.asarray(cheb_b, f32)
    neg_ka8 = -(f32(2.0) * alp - alp * alp) / f32(8.0)
    bf16 = _bf16_dtype()
    return {
        "wenc": np.ascontiguousarray(
            np.asarray(W_enc, f32).reshape(4, 128, 128).transpose(1, 0, 2)
            .astype(bf16)),
        "benc": np.asarray(b_enc, f32).reshape(128, 1).copy(),
        "wap": np.ascontiguousarray(Wap.astype(bf16)),
        "wd": np.ascontiguousarray(Wd.astype(bf16)),
        "d0": np.ascontiguousarray(D0.reshape(128, 1)),
        "nka8": np.ascontiguousarray(neg_ka8.reshape(128, 1)),
        "w12": np.ascontiguousarray(
            np.asarray(lin_W, f32).reshape(2, 128).T.astype(bf16)),  # [128, 2]
        "linb": np.asarray(lin_b, f32).reshape(1, 1).copy(),
        "ident": np.eye(128, dtype=bf16),
    }


def _build_graph(u_pad, e_pad, benc_zero=False, gdeps=None):
    """Build the per-core Bass graph. SPMD: same graph on all 8 cores.

    Engine budget per 512-window chunk (the schedule this graph encodes):
      DVE   S-tree (bf16 2x), clip (4x), max/min trees, xs=r*x, v-chain
      ACT   Square(x), PSUM->SBUF evac with fused +D0 bias, Square(c)
      TEN   per-entry GEMM (Wap/Wd accum), id-matmul sums for Q and ssq
      GPS   gather descriptor generation only (no bulk math: 0.42 eff)
    """
    import concourse.bass as bass
    import concourse.tile as tile
    from concourse import bacc, mybir

    f32 = mybir.dt.float32
    f32r = mybir.dt.float32r
    bf16 = mybir.dt.bfloat16
    i16 = mybir.dt.int16
    AF = mybir.ActivationFunctionType
    OP = mybir.AluOpType

    nb = u_pad // 512          # encoder column tiles
    nt = u_pad // 128          # x transpose tiles
    ng = e_pad // GSZ          # window groups
    ns16 = e_pad // 16

    nc = bacc.Bacc()
    posT_p = nc.declare_dram_parameter("posT", [nb, 128, 4, 512], bf16, False)
    idx_p = nc.declare_dram_parameter("idx", [128, ns16], i16, False)
    wenc_p = nc.declare_dram_parameter("wenc", [128, 4, 128], bf16, False)
    benc_p = nc.declare_dram_parameter("benc", [128, 1], f32, False)
    wap_p = nc.declare_dram_parameter("wap", [128, 128], bf16, False)
    wd_p = nc.declare_dram_parameter("wd", [128, 128], bf16, False)
    d0_p = nc.declare_dram_parameter("d0", [128, 1], f32, False)
    nka8_p = nc.declare_dram_parameter("nka8", [128, 1], f32, False)
    w12_p = nc.declare_dram_parameter("w12", [128, 2], bf16, False)
    linb_p = nc.declare_dram_parameter("linb", [1, 1], f32, False)
    ident_p = nc.declare_dram_parameter("ident", [128, 128], bf16, False)
    out_p = nc.declare_dram_parameter("out", [1, e_pad], f32, True)

    # gather scratch: x rows, bf16, padded so overlapping window reads stay
    # in bounds
    x_dram = nc.dram_tensor("x_scratch", [u_pad + M, 128], bf16)

    with tile.TileContext(nc) as tc:
        with (
            tc.tile_pool(name="consts", bufs=1) as consts,
            # PSUM budget (8 banks of [128, 512] f32):
            #   mm   bufs=2  [128,512]   encoder out / Q-sum / ssq-sum  (2)
            #   pst  bufs=2  [128,4,128] bf16 phase-A transposes        (1)
            #   u    bufs=1  [128,4,512] GEMM groups of 4 entries       (4)
            #   log  bufs=1  [1,512]     logits                         (1)
            tc.tile_pool(name="psum_mm", bufs=2, space="PSUM") as psum_mm,
            tc.tile_pool(name="psum_pst", bufs=1, space="PSUM") as psum_pst,
            tc.tile_pool(name="psum_log", bufs=1, space="PSUM") as psum_log,
            tc.tile_pool(name="psum_u", bufs=2, space="PSUM") as psum_u,
        ):
            # ---- constants ----
            wenc_t = consts.tile([128, 4, 128], bf16)
            nc.scalar.dma_start(out=wenc_t[:, :, :], in_=wenc_p[:, :, :])
            benc_t = consts.tile([128, 1], f32)
            nc.scalar.dma_start(out=benc_t[:, :], in_=benc_p[:, :])
            wap_t = consts.tile([128, 128], bf16)
            nc.scalar.dma_start(out=wap_t[:, :], in_=wap_p[:, :])
            wd_t = consts.tile([128, 128], bf16)
            nc.scalar.dma_start(out=wd_t[:, :], in_=wd_p[:, :])
            d0_t = consts.tile([128, 1], f32)
            nc.scalar.dma_start(out=d0_t[:, :], in_=d0_p[:, :])
            nka8_t = consts.tile([128, 1], f32)
            nc.scalar.dma_start(out=nka8_t[:, :], in_=nka8_p[:, :])
            w12_t = consts.tile([128, 2], bf16)
            nc.scalar.dma_start(out=w12_t[:, :], in_=w12_p[:, :])
            linb_t = consts.tile([1, 1], f32)
            nc.scalar.dma_start(out=linb_t[:, :], in_=linb_p[:, :])
            ident_t = consts.tile([128, 128], bf16)
            nc.scalar.dma_start(out=ident_t[:, :], in_=ident_p[:, :])
            idx_t = consts.tile([128, ns16], i16)
            nc.scalar.dma_start(out=idx_t[:, :], in_=idx_p[:, :])
            eps_t = consts.tile([128, 1], f32)
            nc.vector.memset(eps_t[:, :], EPS)
            from concourse import library_config
            nc.gpsimd.load_library(library_config.mlp)

            # ---- phase A: encoder x = hardtanh(pos @ W_enc + b_enc) ----
            # NOTE: phase-A and phase-B pools are all opened here, in one
            # scope. Nested/sequential pools would reuse released SBUF and
            # the allocator then makes phase-B's first writers (the gathers)
            # wait for ALL of phase A -- a ~35us false serialization.
            x_writes = []
            with (
                tc.tile_pool(name="pos", bufs=3) as pos_pool,
                tc.tile_pool(name="xt", bufs=1) as xt_pool,
                tc.tile_pool(name="stage", bufs=4) as stage_pool,
                tc.tile_pool(name="xe", bufs=4) as xe_pool,
                tc.tile_pool(name="work", bufs=2) as work_pool,
                tc.tile_pool(name="stats", bufs=2) as stats_pool,
                tc.tile_pool(name="fixed", bufs=1) as fixed_pool,
            ):
                xT = xt_pool.tile([128, u_pad], bf16)  # [D, node]
                # zero-pad tail rows first: every gather depends on it, so
                # emitting it before the encoder loop unblocks chunk-0
                # gathers as soon as their node range is written
                zstg = stage_pool.tile([128, 128], bf16, tag="zpad")
                nc.vector.memset(zstg[:, :], 0)
                w = nc.gpsimd.dma_start(out=x_dram[u_pad:u_pad + M, :],
                                        in_=zstg[0:M, :])
                x_writes.append(w)
                for b in range(nb):
                    pos_tile = pos_pool.tile([128, 4, 512], bf16, tag="pos")
                    nc.sync.dma_start(out=pos_tile[:, :, :],
                                      in_=posT_p[b, :, :, :])
                    ps = psum_mm.tile([128, 512], f32, tag="mm", bufs=2)
                    for k in range(4):
                        nc.tensor.matmul(
                            ps[:, :],
                            lhsT=wenc_t[:, k, :],
                            rhs=pos_tile[:, k, :],
                            start=(k == 0),
                            stop=(k == 3),
                        )
                    # (psum + b_enc) clipped to [-1, 1], cast to bf16
                    bs = slice(b * 512, (b + 1) * 512)
                    if benc_zero:
                        nc.vector.tensor_scalar(
                            out=xT[:, bs], in0=ps[:, :],
                            scalar1=1.0, scalar2=-1.0,
                            op0=OP.min, op1=OP.max)
                    else:
                        nc.vector.tensor_scalar(
                            out=xT[:, bs], in0=ps[:, :],
                            scalar1=benc_t[:, 0:1], scalar2=1.0,
                            op0=OP.add, op1=OP.min)
                        nc.vector.tensor_scalar(
                            out=xT[:, bs], in0=xT[:, bs],
                            scalar1=-1.0, scalar2=None, op0=OP.max)
                    # transpose this 512-node batch to rows and write to
                    # DRAM immediately -- issuing the write before later
                    # posT loads keeps the DMA-sem thresholds low so the
                    # gathers can start as soon as their node range is out
                    stg = stage_pool.tile([128, 4, 128], bf16, tag="stage")
                    pst = psum_pst.tile([128, 4, 128], bf16, tag="pst")
                    for j in range(4):
                        t = 4 * b + j
                        nc.tensor.transpose(
                            out=pst[:, j, :],
                            in_=xT[:, t * 128:(t + 1) * 128],
                            identity=ident_t[:, :],
                        )
                    nc.scalar.copy(out=stg[:, :, :], in_=pst[:, :, :])
                    out_ap = bass.AP(
                        tensor=x_dram, offset=b * 512 * 128,
                        ap=[[128, 128], [128 * 128, 4], [1, 128]])
                    w = nc.scalar.dma_start(out=out_ap, in_=stg[:, :, :])
                    x_writes.append(w)

                # ---- phase B: gather windows, transposed ----
                # chunk-major: xeT_c[ch, e, w'] = x[base_{c*GSZ+w'}+e, ch]
                # (SWDGE descriptor ring holds 1024 descriptors -> max 512
                #  indices per dma_gather; per-chunk tiles let gather c+2
                #  overlap compute of chunk c)
                nsc = GSZ // 16
                x_view = bass.AP(tensor=x_dram, offset=0,
                                 ap=[[128, u_pad], [1, M * 128]])
                xes = []
                for c in range(ng):
                    xeT = xe_pool.tile([128, M, GSZ], bf16, tag="xe")
                    xes.append(xeT)
                    _hp = tc.high_priority()
                    _hp.__enter__()
                    g = nc.gpsimd.dma_gather(
                        out_ap=xeT[:, :, :],
                        in_ap=x_view,
                        idxs_ap=idx_t[:, c * nsc:(c + 1) * nsc],
                        num_idxs=GSZ,
                        num_idxs_reg=GSZ,
                        elem_size=M * 128,
                        elem_step=128,
                        transpose=True,
                    )
                    _hp.__exit__(None, None, None)
                    if gdeps is None:
                        deps = x_writes
                    else:
                        deps = x_writes[:1 + gdeps[c]]
                    for w in deps:
                        tile.add_dep_helper(g.ins, w.ins, reason="x_dram RAW")

                def dve_tree(pool, src, op, tagd, out_dt, mid_dt=None):
                    # log-tree over the 8 entry planes on DVE. max/min run
                    # at 2x in bf16; add runs 1x regardless (HW), so the add
                    # tree uses f32 mids for accuracy at the same speed.
                    # Mids are single-buffered scratch (DVE runs serially),
                    # one shared ring per dtype.
                    mid_dt = mid_dt or bf16
                    tm = "f" if mid_dt is f32 else "h"
                    t4 = pool.tile([128, 4, GSZ], mid_dt, tag="t4" + tm,
                                   bufs=1)
                    nc.vector.tensor_tensor(out=t4[:, :, :],
                                            in0=src[:, 0:4, :],
                                            in1=src[:, 4:8, :], op=op)
                    t2 = pool.tile([128, 2, GSZ], mid_dt, tag="t2" + tm,
                                   bufs=1)
                    nc.vector.tensor_tensor(out=t2[:, :, :],
                                            in0=t4[:, 0:2, :],
                                            in1=t4[:, 2:4, :], op=op)
                    dst = pool.tile([128, GSZ], out_dt, tag=tagd)
                    nc.vector.tensor_tensor(out=dst[:, :], in0=t2[:, 0, :],
                                            in1=t2[:, 1, :], op=op)
                    return dst

                # ---- per-chunk pipeline, emitted in WAVEFRONT order ----
                # The Tile scheduler keys heavily on program order per
                # engine; per-chunk sequential emission produced ~25us
                # serialized chunk periods. Emitting stage3(c-2),
                # stage2(c-1), stage1(c) per tick interleaves three chunks
                # in every engine's stream.
                logit_sb = fixed_pool.tile([1, e_pad], f32, tag="lg")
                sig = logit_sb
                st = [dict() for _ in range(ng)]

                def stage1(c):
                    # stats -> r -> xs
                    xeT = xes[c]
                    S = dve_tree(stats_pool, xeT, OP.add, "S", f32,
                                 mid_dt=f32)
                    sq = work_pool.tile([128, M, GSZ], bf16, tag="sq",
                                        bufs=3)
                    nc.scalar.activation(sq[:, :, :], xeT[:, :, :], AF.Square)
                    psQ = psum_mm.tile([128, 512], f32, tag="mm")
                    for e in range(M):
                        nc.tensor.matmul(psQ[:, 0:GSZ], lhsT=ident_t[:, :],
                                         rhs=sq[:, e, :], start=(e == 0),
                                         stop=(e == M - 1))
                    vs = stats_pool.tile([128, GSZ], f32, tag="vs")
                    nc.scalar.activation(vs[:, :], S[:, :], AF.Square)
                    v = stats_pool.tile([128, GSZ], f32, tag="v")
                    nc.vector.scalar_tensor_tensor(
                        out=v[:, :], in0=vs[:, :], scalar=nka8_t[:, 0:1],
                        in1=psQ[:, 0:GSZ], op0=OP.mult, op1=OP.add,
                    )
                    sd = stats_pool.tile([128, GSZ], f32, tag="sd")
                    nc.scalar.activation(sd[:, :], v[:, :], AF.Sqrt,
                                         bias=eps_t[:, 0:1], scale=0.125)
                    r = stats_pool.tile([128, GSZ], f32, tag="r")
                    nc.vector.reciprocal_approx_fast(out=r[:, :],
                                                     in_=sd[:, :])
                    r_bf = stats_pool.tile([128, GSZ], bf16, tag="rb")
                    nc.vector.tensor_copy(out=r_bf[:, :], in_=r[:, :])
                    p_bf = stats_pool.tile([128, GSZ], bf16, tag="p", bufs=3)
                    nc.vector.tensor_tensor(out=p_bf[:, :], in0=r[:, :],
                                            in1=S[:, :], op=OP.mult)
                    xs = work_pool.tile([128, M, GSZ], bf16, tag="xs", bufs=3)
                    r_ap = r_bf[:, :]
                    r_bc = bass.AP(tensor=r_ap.tensor, offset=r_ap.offset,
                                   ap=[r_ap.ap[0], [0, 4], r_ap.ap[-1]])
                    nc.vector.tensor_tensor(out=xs[:, 0:4, :],
                                            in0=xeT[:, 0:4, :],
                                            in1=r_bc, op=OP.mult)
                    nc.vector.tensor_tensor(out=xs[:, 4:8, :],
                                            in0=xeT[:, 4:8, :],
                                            in1=r_bc, op=OP.mult)
                    st[c].update(xs=xs, p_bf=p_bf)

                def stage2(c):
                    # per-entry GEMM, 4 double-buffered 2-bank groups; ACT
                    # evac with fused +D0
                    xs, p_bf = st[c]["xs"], st[c]["p_bf"]
                    u_sb = work_pool.tile([128, M, GSZ], bf16, tag="u",
                                          bufs=3)
                    for gq in range(4):
                        # [128, 2, 512] so each entry owns a whole PSUM bank
                        # (start=True resets bank-granular accumulation)
                        psU = psum_u.tile([128, 2, 512], f32, tag="u")
                        for j in range(2):
                            nc.tensor.matmul(psU[:, j, 0:GSZ],
                                             lhsT=wap_t[:, :],
                                             rhs=xs[:, 2 * gq + j, :],
                                             start=True, stop=False)
                        for j in range(2):
                            nc.tensor.matmul(psU[:, j, 0:GSZ],
                                             lhsT=wd_t[:, :],
                                             rhs=p_bf[:, :],
                                             start=False, stop=True)
                        nc.scalar.activation(u_sb[:, 2 * gq:2 * gq + 2, :],
                                             psU[:, :, 0:GSZ], AF.Identity,
                                             bias=d0_t[:, 0:1], scale=1.0)
                    st[c]["u_sb"] = u_sb

                def stage3(c):
                    # pools: ymm = clip(max) - clip(min); ssq = sum clip^2
                    cs = slice(c * GSZ, (c + 1) * GSZ)
                    u_sb = st[c]["u_sb"]
                    c_t = work_pool.tile([128, M, GSZ], bf16, tag="c", bufs=1)
                    nc.vector.tensor_scalar(out=c_t[:, :, :],
                                            in0=u_sb[:, :, :],
                                            scalar1=1.0, scalar2=-1.0,
                                            op0=OP.min, op1=OP.max)
                    csq = work_pool.tile([128, M, GSZ], bf16, tag="sq",
                                         bufs=3)
                    nc.scalar.activation(csq[:, :, :], c_t[:, :, :],
                                         AF.Square)
                    psR = psum_mm.tile([128, 512], f32, tag="mm")
                    for e in range(M):
                        nc.tensor.matmul(psR[:, 0:GSZ], lhsT=ident_t[:, :],
                                         rhs=csq[:, e, :], start=(e == 0),
                                         stop=(e == M - 1))
                    rms = stats_pool.tile([128, GSZ], bf16, tag="rms")
                    nc.scalar.activation(rms[:, :], psR[:, 0:GSZ], AF.Sqrt,
                                         scale=0.125)
                    umax = dve_tree(stats_pool, u_sb, OP.max, "ux", bf16)
                    umin = dve_tree(stats_pool, u_sb, OP.min, "un", bf16)
                    nc.vector.tensor_scalar(out=umax[:, :], in0=umax[:, :],
                                            scalar1=1.0, scalar2=-1.0,
                                            op0=OP.min, op1=OP.max)
                    nc.vector.tensor_scalar(out=umin[:, :], in0=umin[:, :],
                                            scalar1=1.0, scalar2=-1.0,
                                            op0=OP.min, op1=OP.max)
                    nc.vector.tensor_tensor(out=umax[:, :], in0=umax[:, :],
                                            in1=umin[:, :], op=OP.subtract)
                    psl = psum_log.tile([1, 512], f32, tag="log")
                    nc.tensor.matmul(psl[:, 0:GSZ], lhsT=w12_t[:, 0:1],
                                     rhs=umax[:, :], start=True, stop=False)
                    nc.tensor.matmul(psl[:, 0:GSZ], lhsT=w12_t[:, 1:2],
                                     rhs=rms[:, :], start=False, stop=True)
                    nc.scalar.activation(logit_sb[:, cs], psl[:, 0:GSZ],
                                         AF.Identity)
                    st[c].clear()

                for t in range(ng + 3):
                    if t >= 3:
                        stage3(t - 3)
                    if t >= 2 and t - 2 < ng:
                        stage2(t - 2)
                    if t < ng:
                        stage1(t)
                nc.scalar.activation(sig[:, :], logit_sb[:, :], AF.Sigmoid,
                                     bias=linb_t[0:1, 0:1], scale=1.0)
                nc.gpsimd.dma_start(out=out_p[:, :], in_=sig[:, :])

    nc.finalize()
    return nc


def _np_fallback(pos_set, W_enc, b_enc, gn_gamma, gn_beta, gn_alpha, cheb_W,
                 cheb_b, lin_W, lin_b, members, edge_index, batch):
    """Pure-numpy general path (only used if the expected input structure is
    absent; inputs from setup_inputs always take the device path)."""
    f32 = np.float32
    E = members.shape[0]
    num_entries = members.size
    x = np.clip(pos_set @ W_enc + b_enc, -1.0, 1.0).astype(f32)
    xe = x[members.reshape(-1)]
    cnt = np.zeros(E, f32)
    np.add.at(cnt, batch, 1.0)
    mean = np.zeros((E, x.shape[1]), f32)
    np.add.at(mean, batch, xe)
    mean /= cnt[:, None]
    ctr = xe - gn_alpha * mean[batch]
    var = np.zeros((E, x.shape[1]), f32)
    np.add.at(var, batch, ctr * ctr)
    var /= cnt[:, None]
    xe = gn_gamma * ctr / np.sqrt(var + EPS)[batch] + gn_beta
    src, dst = edge_index[0], edge_index[1]
    deg = np.zeros(num_entries, f32)
    np.add.at(deg, dst, 1.0)
    w = -1.0 / np.sqrt(deg[src] * deg[dst])

    def lhat(h):
        out = np.zeros_like(h)
        np.add.at(out, dst, w[:, None] * h[src])
        return out

    tx0 = xe
    tx1 = lhat(tx0)
    out = tx0 @ cheb_W[0] + tx1 @ cheb_W[1]
    tkm1, tkm2 = tx1, tx0
    for k in range(2, cheb_W.shape[0]):
        tk = 2.0 * lhat(tkm1) - tkm2
        out = out + tk @ cheb_W[k]
        tkm1, tkm2 = tk, tkm1
    h = np.clip(out + cheb_b, -1.0, 1.0)
    ymax = np.full((E, h.shape[1]), -np.inf, f32)
    ymin = np.full((E, h.shape[1]), np.inf, f32)
    np.maximum.at(ymax, batch, h)
    np.minimum.at(ymin, batch, h)
    ynorm = np.zeros((E, h.shape[1]), f32)
    np.add.at(ynorm, batch, h * h)
    ynorm = np.sqrt(ynorm / cnt[:, None])
    y = np.concatenate([ymax - ymin, ynorm], axis=1)
    logits = y @ lin_W + lin_b
    return (1.0 / (1.0 + np.exp(-logits))).squeeze(-1).astype(f32)


def _has_window_structure(members, edge_index, batch):
    E, Mm = members.shape
    if Mm != M:
        return False
    base = members[:, 0]
    if not (members == base[:, None] + np.arange(M, dtype=members.dtype)).all():
        return False
    if not (batch == np.repeat(np.arange(E, dtype=batch.dtype), M)).all():
        return False
    row, col = np.where(~np.eye(M, dtype=bool))
    offs = np.arange(E, dtype=np.int64)[:, None] * M
    ei = np.stack([(offs + row[None, :]).ravel(), (offs + col[None, :]).ravel()])
    return (edge_index == ei).all()


def kernel(pos_set, W_enc, b_enc, gn_gamma, gn_beta, gn_alpha, cheb_W, cheb_b,
           lin_W, lin_b, members, edge_index, batch):
    pos_set = np.asarray(pos_set, np.float32)
    members = np.asarray(members)
    edge_index = np.asarray(edge_index)
    batch = np.asarray(batch)
    if not _has_window_structure(members, edge_index, batch):
        return _np_fallback(
            pos_set, np.asarray(W_enc, np.float32),
            np.asarray(b_enc, np.float32), np.asarray(gn_gamma, np.float32),
            np.asarray(gn_beta, np.float32), np.asarray(gn_alpha, np.float32),
            np.asarray(cheb_W, np.float32), np.asarray(cheb_b, np.float32),
            np.asarray(lin_W, np.float32), np.asarray(lin_b, np.float32),
            members, edge_index, batch)

    N = pos_set.shape[0]
    E = members.shape[0]
    base = members[:, 0].astype(np.int64)
    node_span = (N + N_CORES - 1) // N_CORES                # 6250
    u_pad = ((node_span + M + 511) // 512 + 1) * 512        # 6656 for N=50000
    # quantile split: sort windows by base, give each core an equal count.
    # Cores then own contiguous (narrow) base bands; sorted bases inside a
    # core give gather chunks narrow node ranges -> partial x-write deps
    # (pipelining). Falls back to equal-node split if a band's node span
    # exceeds u_pad.
    order = np.argsort(base, kind="stable")
    ecnt = (E + N_CORES - 1) // N_CORES
    counts = np.array([min(ecnt, E - c * ecnt) for c in range(N_CORES)])
    offs_pre = np.concatenate([[0], np.cumsum(counts)])
    node_lo = np.zeros(N_CORES, np.int64)
    ok = True
    for c in range(N_CORES):
        ids = order[offs_pre[c]:offs_pre[c + 1]]
        if ids.size == 0:
            node_lo[c] = 0
            continue
        node_lo[c] = base[ids[0]]
        if base[ids[-1]] + M - node_lo[c] > u_pad:
            ok = False
            break
    if not ok:
        core_of = np.minimum(base // node_span, N_CORES - 1)
        order = np.argsort(base, kind="stable")
        counts = np.bincount(core_of, minlength=N_CORES)
        offs_pre = np.concatenate([[0], np.cumsum(counts)])
        node_lo = np.arange(N_CORES, dtype=np.int64) * node_span
    e_pad = max(GSZ, int(math.ceil(counts.max() / GSZ)) * GSZ)

    benc_zero = bool(np.all(np.asarray(b_enc) == 0.0))
    ng_ = e_pad // GSZ
    nwb_ = u_pad // 512
    # per-chunk: how many 512-node x-write batches the gather depends on
    # (max over cores, from the actual window bases)
    gdeps = []
    for c in range(ng_):
        mx = 0
        for cc in range(N_CORES):
            ids = order[offs_pre[cc] + c * GSZ:
                        min(offs_pre[cc] + (c + 1) * GSZ, offs_pre[cc + 1])]
            if ids.size:
                mx = max(mx, int((base[ids] - node_lo[cc]).max()))
        gdeps.append(min(nwb_, (mx + M + 511) // 512))
    gdeps = tuple(gdeps)
    key = (u_pad, e_pad, benc_zero, gdeps)
    if key not in _GRAPH_CACHE:
        _GRAPH_CACHE[key] = _build_graph(u_pad, e_pad, benc_zero, gdeps)
    nc = _GRAPH_CACHE[key]

    shared = _fold_weights(W_enc, b_enc, gn_gamma, gn_beta, gn_alpha, cheb_W,
                           cheb_b, lin_W, lin_b)
    nb = u_pad // 512
    ns16 = e_pad // 16

    in_maps = []
    offs = offs_pre
    for c in range(N_CORES):
        lo = int(node_lo[c])
        sl = pos_set[lo:min(lo + u_pad, N)]
        if sl.shape[0] < u_pad:
            sl = np.concatenate(
                [sl, np.zeros((u_pad - sl.shape[0], F), np.float32)], 0)
        # posT[b, p, k, u'] = sl[512b+u', 128k+p]
        posT = np.ascontiguousarray(
            sl.reshape(nb, 512, 4, 128).transpose(0, 3, 2, 1)
            .astype(_bf16_dtype()))
        ids = order[offs[c]:offs[c + 1]]
        loc = (base[ids] - lo).astype(np.int64)
        idx = np.zeros(e_pad, np.int16)
        idx[:loc.size] = loc.astype(np.int16)
        # wrapped layout: element i lives at [i % 16, i // 16], replicated
        # across the eight 16-partition groups
        w16 = idx.reshape(ns16, 16).T           # [16, ns16]
        m = dict(shared)
        m["posT"] = posT
        m["idx"] = np.ascontiguousarray(np.tile(w16, (8, 1)))
        in_maps.append(m)

    import os

    from concourse.bass_utils import run_bass_kernel_spmd

    trace = bool(os.environ.get("CHESHIRE_TRACE"))
    res = run_bass_kernel_spmd(nc, in_maps, core_ids=list(range(N_CORES)),
                               trace=trace)
    global LAST_EXEC_NS, LAST_RESULT
    LAST_EXEC_NS = res.exec_time_ns
    LAST_RESULT = res
    out_full = np.zeros(E, np.float32)
    for c in range(N_CORES):
        ids = order[offs[c]:offs[c + 1]]
        vals = np.asarray(res.results[c]["out"], np.float32).reshape(-1)
        out_full[ids] = vals[:ids.size]
    return out_full



# revision 53
# speedup vs baseline: 1.3699x; 1.3699x over previous
"""CHESHIRE hyperedge link predictor on 8 Trainium2 NeuronCores.

Structure exploited (verified at runtime):
  - members[e] = base[e] + arange(8): each hyperedge is a contiguous
    8-node window -> sorting hyperedges by base makes the whole problem
    embarrassingly parallel across a node-range partition (no collectives).
  - edge_index is the full directed 8-clique per hyperedge -> deg == 7,
    w == -1/7, so Lhat(h) = (h - sum(h))/7 and the K=3 ChebConv folds into
    a single per-entry GEMM plus a per-window GEMM:
        u_i = (r * x_i) @ Wap + (r * S) @ Wd + D0
    with r = 1/sqrt(var+eps) (GraphNorm fold), S = window sum of x.
  - clip commutes with max/min pools; clip(u)^2 == min(u^2, 1) for the rms
    pool, so hardtanh is applied only to pooled quantities.

Per-core device pipeline:
  encoder GEMM (f32r) -> +bias, hardtanh -> x to DRAM (bf16) -> dma_gather
  (transpose mode) produces xeT[ch, entry, window] directly -> window
  stats via free-axis reduces -> r, p=r*S -> per-entry GEMM with the
  per-window term accumulated into the same PSUM -> pools via free-axis
  reduces -> logits GEMM -> sigmoid -> out.
"""

import math

import numpy as np

N_CORES = 8
M = 8          # nodes per hyperedge
D = 128        # embedding dim
F = 512        # input feature dim
EPS = 1e-5
GSZ = 256      # windows per PSUM group

_GRAPH_CACHE = {}
LAST_EXEC_NS = None
LAST_RESULT = None


def _bf16_dtype():
    import ml_dtypes

    return np.dtype(ml_dtypes.bfloat16)


def _fold_weights(W_enc, b_enc, gn_gamma, gn_beta, gn_alpha, cheb_W, cheb_b,
                  lin_W, lin_b):
    f32 = np.float32
    W0, W1, W2 = (np.asarray(cheb_W[i], f32) for i in range(3))
    gam = np.asarray(gn_gamma, f32)
    bet = np.asarray(gn_beta, f32)
    alp = np.asarray(gn_alpha, f32)
    Wa = W0 + W1 / f32(7.0) - f32(47.0 / 49.0) * W2
    Wb = -W1 / f32(7.0) + f32(12.0 / 49.0) * W2
    Wap = gam[:, None] * Wa
    Wd = -(gam * alp / f32(8.0))[:, None] * (Wa + f32(8.0) * Wb) + gam[:, None] * Wb
    D0 = bet @ Wa + f32(8.0) * (bet @ Wb) + np.asarray(cheb_b, f32)
    neg_ka8 = -(f32(2.0) * alp - alp * alp) / f32(8.0)
    bf16 = _bf16_dtype()
    return {
        "wenc": np.ascontiguousarray(
            np.asarray(W_enc, f32).reshape(4, 128, 128).transpose(1, 0, 2)
            .astype(bf16)),
        "benc": np.asarray(b_enc, f32).reshape(128, 1).copy(),
        "wap": np.ascontiguousarray(Wap.astype(bf16)),
        "wd": np.ascontiguousarray(Wd.astype(bf16)),
        "d0": np.ascontiguousarray(D0.reshape(128, 1)),
        "nka8": np.ascontiguousarray(neg_ka8.reshape(128, 1)),
        "w12": np.ascontiguousarray(
            np.asarray(lin_W, f32).reshape(2, 128).T.astype(bf16)),  # [128, 2]
        "linb": np.asarray(lin_b, f32).reshape(1, 1).copy(),
        "ident": np.eye(128, dtype=bf16),
    }


def _build_graph(u_pad, e_pad, benc_zero=False, gdeps=None):
    """Build the per-core Bass graph. SPMD: same graph on all 8 cores.

    Engine budget per 512-window chunk (the schedule this graph encodes):
      DVE   S-tree (bf16 2x), clip (4x), max/min trees, xs=r*x, v-chain
      ACT   Square(x), PSUM->SBUF evac with fused +D0 bias, Square(c)
      TEN   per-entry GEMM (Wap/Wd accum), id-matmul sums for Q and ssq
      GPS   gather descriptor generation only (no bulk math: 0.42 eff)
    """
    import concourse.bass as bass
    import concourse.tile as tile
    from concourse import bacc, mybir

    f32 = mybir.dt.float32
    f32r = mybir.dt.float32r
    bf16 = mybir.dt.bfloat16
    i16 = mybir.dt.int16
    AF = mybir.ActivationFunctionType
    OP = mybir.AluOpType

    nb = u_pad // 512          # encoder column tiles
    nt = u_pad // 128          # x transpose tiles
    ng = e_pad // GSZ          # window groups
    ns16 = e_pad // 16

    nc = bacc.Bacc()
    posT_p = nc.declare_dram_parameter("posT", [nb, 128, 4, 512], bf16, False)
    idx_p = nc.declare_dram_parameter("idx", [128, ns16], i16, False)
    wenc_p = nc.declare_dram_parameter("wenc", [128, 4, 128], bf16, False)
    benc_p = nc.declare_dram_parameter("benc", [128, 1], f32, False)
    wap_p = nc.declare_dram_parameter("wap", [128, 128], bf16, False)
    wd_p = nc.declare_dram_parameter("wd", [128, 128], bf16, False)
    d0_p = nc.declare_dram_parameter("d0", [128, 1], f32, False)
    nka8_p = nc.declare_dram_parameter("nka8", [128, 1], f32, False)
    w12_p = nc.declare_dram_parameter("w12", [128, 2], bf16, False)
    linb_p = nc.declare_dram_parameter("linb", [1, 1], f32, False)
    ident_p = nc.declare_dram_parameter("ident", [128, 128], bf16, False)
    out_p = nc.declare_dram_parameter("out", [1, e_pad], f32, True)

    # gather scratch: x rows, bf16, padded so overlapping window reads stay
    # in bounds
    x_dram = nc.dram_tensor("x_scratch", [u_pad + M, 128], bf16)

    with tile.TileContext(nc) as tc:
        with (
            tc.tile_pool(name="consts", bufs=1) as consts,
            # PSUM budget (8 banks of [128, 512] f32):
            #   mm   bufs=2  [128,512]   encoder out / Q-sum / ssq-sum  (2)
            #   pst  bufs=2  [128,4,128] bf16 phase-A transposes        (1)
            #   u    bufs=1  [128,4,512] GEMM groups of 4 entries       (4)
            #   log  bufs=1  [1,512]     logits                         (1)
            tc.tile_pool(name="psum_mm", bufs=2, space="PSUM") as psum_mm,
            tc.tile_pool(name="psum_pst", bufs=1, space="PSUM") as psum_pst,
            tc.tile_pool(name="psum_log", bufs=1, space="PSUM") as psum_log,
            tc.tile_pool(name="psum_u", bufs=2, space="PSUM") as psum_u,
        ):
            # ---- constants ----
            wenc_t = consts.tile([128, 4, 128], bf16)
            nc.scalar.dma_start(out=wenc_t[:, :, :], in_=wenc_p[:, :, :])
            benc_t = consts.tile([128, 1], f32)
            nc.scalar.dma_start(out=benc_t[:, :], in_=benc_p[:, :])
            wap_t = consts.tile([128, 128], bf16)
            nc.scalar.dma_start(out=wap_t[:, :], in_=wap_p[:, :])
            wd_t = consts.tile([128, 128], bf16)
            nc.scalar.dma_start(out=wd_t[:, :], in_=wd_p[:, :])
            d0_t = consts.tile([128, 1], f32)
            nc.scalar.dma_start(out=d0_t[:, :], in_=d0_p[:, :])
            nka8_t = consts.tile([128, 1], f32)
            nc.scalar.dma_start(out=nka8_t[:, :], in_=nka8_p[:, :])
            w12_t = consts.tile([128, 2], bf16)
            nc.scalar.dma_start(out=w12_t[:, :], in_=w12_p[:, :])
            linb_t = consts.tile([1, 1], f32)
            nc.scalar.dma_start(out=linb_t[:, :], in_=linb_p[:, :])
            ident_t = consts.tile([128, 128], bf16)
            nc.scalar.dma_start(out=ident_t[:, :], in_=ident_p[:, :])
            idx_t = consts.tile([128, ns16], i16)
            nc.scalar.dma_start(out=idx_t[:, :], in_=idx_p[:, :])
            eps_t = consts.tile([128, 1], f32)
            nc.vector.memset(eps_t[:, :], EPS)
            from concourse import library_config
            nc.gpsimd.load_library(library_config.mlp)

            # ---- phase A: encoder x = hardtanh(pos @ W_enc + b_enc) ----
            # NOTE: phase-A and phase-B pools are all opened here, in one
            # scope. Nested/sequential pools would reuse released SBUF and
            # the allocator then makes phase-B's first writers (the gathers)
            # wait for ALL of phase A -- a ~35us false serialization.
            x_writes = []
            with (
                tc.tile_pool(name="pos", bufs=3) as pos_pool,
                tc.tile_pool(name="xt", bufs=1) as xt_pool,
                tc.tile_pool(name="stage", bufs=4) as stage_pool,
                tc.tile_pool(name="xe", bufs=4) as xe_pool,
                tc.tile_pool(name="work", bufs=2) as work_pool,
                tc.tile_pool(name="stats", bufs=2) as stats_pool,
                tc.tile_pool(name="fixed", bufs=1) as fixed_pool,
            ):
                xT = xt_pool.tile([128, u_pad], bf16)  # [D, node]
                # zero-pad tail rows first: every gather depends on it, so
                # emitting it before the encoder loop unblocks chunk-0
                # gathers as soon as their node range is written
                zstg = stage_pool.tile([128, 128], bf16, tag="zpad")
                nc.vector.memset(zstg[:, :], 0)
                w = nc.gpsimd.dma_start(out=x_dram[u_pad:u_pad + M, :],
                                        in_=zstg[0:M, :])
                x_writes.append(w)
                for b in range(nb):
                    pos_tile = pos_pool.tile([128, 4, 512], bf16, tag="pos")
                    nc.sync.dma_start(out=pos_tile[:, :, :],
                                      in_=posT_p[b, :, :, :])
                    ps = psum_mm.tile([128, 512], f32, tag="mm", bufs=2)
                    for k in range(4):
                        nc.tensor.matmul(
                            ps[:, :],
                            lhsT=wenc_t[:, k, :],
                            rhs=pos_tile[:, k, :],
                            start=(k == 0),
                            stop=(k == 3),
                        )
                    # (psum + b_enc) clipped to [-1, 1], cast to bf16
                    bs = slice(b * 512, (b + 1) * 512)
                    if benc_zero:
                        nc.vector.tensor_scalar(
                            out=xT[:, bs], in0=ps[:, :],
                            scalar1=1.0, scalar2=-1.0,
                            op0=OP.min, op1=OP.max)
                    else:
                        nc.vector.tensor_scalar(
                            out=xT[:, bs], in0=ps[:, :],
                            scalar1=benc_t[:, 0:1], scalar2=1.0,
                            op0=OP.add, op1=OP.min)
                        nc.vector.tensor_scalar(
                            out=xT[:, bs], in0=xT[:, bs],
                            scalar1=-1.0, scalar2=None, op0=OP.max)
                    # transpose this 512-node batch to rows and write to
                    # DRAM immediately -- issuing the write before later
                    # posT loads keeps the DMA-sem thresholds low so the
                    # gathers can start as soon as their node range is out
                    stg = stage_pool.tile([128, 4, 128], bf16, tag="stage")
                    pst = psum_pst.tile([128, 4, 128], bf16, tag="pst")
                    for j in range(4):
                        t = 4 * b + j
                        nc.tensor.transpose(
                            out=pst[:, j, :],
                            in_=xT[:, t * 128:(t + 1) * 128],
                            identity=ident_t[:, :],
                        )
                    nc.scalar.copy(out=stg[:, :, :], in_=pst[:, :, :])
                    out_ap = bass.AP(
                        tensor=x_dram, offset=b * 512 * 128,
                        ap=[[128, 128], [128 * 128, 4], [1, 128]])
                    w = nc.scalar.dma_start(out=out_ap, in_=stg[:, :, :])
                    x_writes.append(w)

                # ---- phase B: gather windows, transposed ----
                # chunk-major: xeT_c[ch, e, w'] = x[base_{c*GSZ+w'}+e, ch]
                # (SWDGE descriptor ring holds 1024 descriptors -> max 512
                #  indices per dma_gather; per-chunk tiles let gather c+2
                #  overlap compute of chunk c)
                nsc = GSZ // 16
                x_view = bass.AP(tensor=x_dram, offset=0,
                                 ap=[[128, u_pad], [1, M * 128]])
                xes = []
                for c in range(ng):
                    xeT = xe_pool.tile([128, M, GSZ], bf16, tag="xe")
                    xes.append(xeT)
                    _hp = tc.high_priority()
                    _hp.__enter__()
                    g = nc.gpsimd.dma_gather(
                        out_ap=xeT[:, :, :],
                        in_ap=x_view,
                        idxs_ap=idx_t[:, c * nsc:(c + 1) * nsc],
                        num_idxs=GSZ,
                        num_idxs_reg=GSZ,
                        elem_size=M * 128,
                        elem_step=128,
                        transpose=True,
                    )
                    _hp.__exit__(None, None, None)
                    if gdeps is None:
                        deps = x_writes
                    else:
                        deps = x_writes[:1 + gdeps[c]]
                    for w in deps:
                        tile.add_dep_helper(g.ins, w.ins, reason="x_dram RAW")

                def dve_tree(pool, src, op, tagd, out_dt, mid_dt=None):
                    # log-tree over the 8 entry planes on DVE. max/min run
                    # at 2x in bf16; add runs 1x regardless (HW), so the add
                    # tree uses f32 mids for accuracy at the same speed.
                    # Mids are single-buffered scratch (DVE runs serially),
                    # one shared ring per dtype.
                    mid_dt = mid_dt or bf16
                    tm = "f" if mid_dt is f32 else "h"
                    t4 = pool.tile([128, 4, GSZ], mid_dt, tag="t4" + tm,
                                   bufs=1)
                    nc.vector.tensor_tensor(out=t4[:, :, :],
                                            in0=src[:, 0:4, :],
                                            in1=src[:, 4:8, :], op=op)
                    t2 = pool.tile([128, 2, GSZ], mid_dt, tag="t2" + tm,
                                   bufs=1)
                    nc.vector.tensor_tensor(out=t2[:, :, :],
                                            in0=t4[:, 0:2, :],
                                            in1=t4[:, 2:4, :], op=op)
                    dst = pool.tile([128, GSZ], out_dt, tag=tagd)
                    nc.vector.tensor_tensor(out=dst[:, :], in0=t2[:, 0, :],
                                            in1=t2[:, 1, :], op=op)
                    return dst

                # ---- per-chunk pipeline, emitted in WAVEFRONT order ----
                # The Tile scheduler keys heavily on program order per
                # engine; per-chunk sequential emission produced ~25us
                # serialized chunk periods. Emitting stage3(c-2),
                # stage2(c-1), stage1(c) per tick interleaves three chunks
                # in every engine's stream.
                logit_sb = fixed_pool.tile([1, e_pad], f32, tag="lg")
                sig = logit_sb
                st = [dict() for _ in range(ng)]

                def stage1(c):
                    # stats -> r -> xs
                    xeT = xes[c]
                    S = dve_tree(stats_pool, xeT, OP.add, "S", f32,
                                 mid_dt=f32)
                    sq = work_pool.tile([128, M, GSZ], bf16, tag="sq",
                                        bufs=3)
                    nc.scalar.activation(sq[:, :, :], xeT[:, :, :], AF.Square)
                    psQ = psum_mm.tile([128, 512], f32, tag="mm")
                    for e in range(M):
                        nc.tensor.matmul(psQ[:, 0:GSZ], lhsT=ident_t[:, :],
                                         rhs=sq[:, e, :], start=(e == 0),
                                         stop=(e == M - 1))
                    vs = stats_pool.tile([128, GSZ], f32, tag="vs")
                    nc.scalar.activation(vs[:, :], S[:, :], AF.Square)
                    v = stats_pool.tile([128, GSZ], f32, tag="v")
                    nc.vector.scalar_tensor_tensor(
                        out=v[:, :], in0=vs[:, :], scalar=nka8_t[:, 0:1],
                        in1=psQ[:, 0:GSZ], op0=OP.mult, op1=OP.add,
                    )
                    sd = stats_pool.tile([128, GSZ], f32, tag="sd")
                    nc.scalar.activation(sd[:, :], v[:, :], AF.Sqrt,
                                         bias=eps_t[:, 0:1], scale=0.125)
                    r = stats_pool.tile([128, GSZ], f32, tag="r")
                    nc.vector.reciprocal_approx_fast(out=r[:, :],
                                                     in_=sd[:, :])
                    r_bf = stats_pool.tile([128, GSZ], bf16, tag="rb")
                    nc.vector.tensor_copy(out=r_bf[:, :], in_=r[:, :])
                    p_bf = stats_pool.tile([128, GSZ], bf16, tag="p", bufs=3)
                    nc.vector.tensor_tensor(out=p_bf[:, :], in0=r[:, :],
                                            in1=S[:, :], op=OP.mult)
                    xs = work_pool.tile([128, M, GSZ], bf16, tag="xs", bufs=3)
                    r_ap = r_bf[:, :]
                    r_bc = bass.AP(tensor=r_ap.tensor, offset=r_ap.offset,
                                   ap=[r_ap.ap[0], [0, 4], r_ap.ap[-1]])
                    nc.vector.tensor_tensor(out=xs[:, 0:4, :],
                                            in0=xeT[:, 0:4, :],
                                            in1=r_bc, op=OP.mult)
                    nc.vector.tensor_tensor(out=xs[:, 4:8, :],
                                            in0=xeT[:, 4:8, :],
                                            in1=r_bc, op=OP.mult)
                    st[c].update(xs=xs, p_bf=p_bf)

                def stage2(c):
                    # per-entry GEMM, 4 double-buffered 2-bank groups; ACT
                    # evac with fused +D0
                    xs, p_bf = st[c]["xs"], st[c]["p_bf"]
                    u_sb = work_pool.tile([128, M, GSZ], bf16, tag="u",
                                          bufs=3)
                    for gq in range(4):
                        # [128, 2, 512] so each entry owns a whole PSUM bank
                        # (start=True resets bank-granular accumulation)
                        psU = psum_u.tile([128, 2, 512], f32, tag="u")
                        for j in range(2):
                            nc.tensor.matmul(psU[:, j, 0:GSZ],
                                             lhsT=wap_t[:, :],
                                             rhs=xs[:, 2 * gq + j, :],
                                             start=True, stop=False)
                        for j in range(2):
                            nc.tensor.matmul(psU[:, j, 0:GSZ],
                                             lhsT=wd_t[:, :],
                                             rhs=p_bf[:, :],
                                             start=False, stop=True)
                        nc.scalar.activation(u_sb[:, 2 * gq:2 * gq + 2, :],
                                             psU[:, :, 0:GSZ], AF.Identity,
                                             bias=d0_t[:, 0:1], scale=1.0)
                    st[c]["u_sb"] = u_sb

                def stage3(c):
                    # pools: ymm = clip(max) - clip(min); ssq = sum clip^2
                    cs = slice(c * GSZ, (c + 1) * GSZ)
                    u_sb = st[c]["u_sb"]
                    c_t = work_pool.tile([128, M, GSZ], bf16, tag="c", bufs=1)
                    nc.vector.tensor_scalar(out=c_t[:, :, :],
                                            in0=u_sb[:, :, :],
                                            scalar1=1.0, scalar2=-1.0,
                                            op0=OP.min, op1=OP.max)
                    csq = work_pool.tile([128, M, GSZ], bf16, tag="sq",
                                         bufs=3)
                    nc.scalar.activation(csq[:, :, :], c_t[:, :, :],
                                         AF.Square)
                    psR = psum_mm.tile([128, 512], f32, tag="mm")
                    for e in range(M):
                        nc.tensor.matmul(psR[:, 0:GSZ], lhsT=ident_t[:, :],
                                         rhs=csq[:, e, :], start=(e == 0),
                                         stop=(e == M - 1))
                    rms = stats_pool.tile([128, GSZ], bf16, tag="rms")
                    nc.scalar.activation(rms[:, :], psR[:, 0:GSZ], AF.Sqrt,
                                         scale=0.125)
                    umax = dve_tree(stats_pool, u_sb, OP.max, "ux", bf16)
                    umin = dve_tree(stats_pool, u_sb, OP.min, "un", bf16)
                    nc.vector.tensor_scalar(out=umax[:, :], in0=umax[:, :],
                                            scalar1=1.0, scalar2=-1.0,
                                            op0=OP.min, op1=OP.max)
                    nc.vector.tensor_scalar(out=umin[:, :], in0=umin[:, :],
                                            scalar1=1.0, scalar2=-1.0,
                                            op0=OP.min, op1=OP.max)
                    nc.vector.tensor_tensor(out=umax[:, :], in0=umax[:, :],
                                            in1=umin[:, :], op=OP.subtract)
                    psl = psum_log.tile([1, 512], f32, tag="log")
                    nc.tensor.matmul(psl[:, 0:GSZ], lhsT=w12_t[:, 0:1],
                                     rhs=umax[:, :], start=True, stop=False)
                    nc.tensor.matmul(psl[:, 0:GSZ], lhsT=w12_t[:, 1:2],
                                     rhs=rms[:, :], start=False, stop=True)
                    nc.scalar.activation(logit_sb[:, cs], psl[:, 0:GSZ],
                                         AF.Identity)
                    st[c].clear()

                for t in range(ng + 3):
                    if t >= 3:
                        stage3(t - 3)
                    if t >= 2 and t - 2 < ng:
                        stage2(t - 2)
                    if t < ng:
                        stage1(t)
                nc.scalar.activation(sig[:, :], logit_sb[:, :], AF.Sigmoid,
                                     bias=linb_t[0:1, 0:1], scale=1.0)
                nc.gpsimd.dma_start(out=out_p[:, :], in_=sig[:, :])

    nc.finalize()
    return nc


def _np_fallback(pos_set, W_enc, b_enc, gn_gamma, gn_beta, gn_alpha, cheb_W,
                 cheb_b, lin_W, lin_b, members, edge_index, batch):
    """Pure-numpy general path (only used if the expected input structure is
    absent; inputs from setup_inputs always take the device path)."""
    f32 = np.float32
    E = members.shape[0]
    num_entries = members.size
    x = np.clip(pos_set @ W_enc + b_enc, -1.0, 1.0).astype(f32)
    xe = x[members.reshape(-1)]
    cnt = np.zeros(E, f32)
    np.add.at(cnt, batch, 1.0)
    mean = np.zeros((E, x.shape[1]), f32)
    np.add.at(mean, batch, xe)
    mean /= cnt[:, None]
    ctr = xe - gn_alpha * mean[batch]
    var = np.zeros((E, x.shape[1]), f32)
    np.add.at(var, batch, ctr * ctr)
    var /= cnt[:, None]
    xe = gn_gamma * ctr / np.sqrt(var + EPS)[batch] + gn_beta
    src, dst = edge_index[0], edge_index[1]
    deg = np.zeros(num_entries, f32)
    np.add.at(deg, dst, 1.0)
    w = -1.0 / np.sqrt(deg[src] * deg[dst])

    def lhat(h):
        out = np.zeros_like(h)
        np.add.at(out, dst, w[:, None] * h[src])
        return out

    tx0 = xe
    tx1 = lhat(tx0)
    out = tx0 @ cheb_W[0] + tx1 @ cheb_W[1]
    tkm1, tkm2 = tx1, tx0
    for k in range(2, cheb_W.shape[0]):
        tk = 2.0 * lhat(tkm1) - tkm2
        out = out + tk @ cheb_W[k]
        tkm1, tkm2 = tk, tkm1
    h = np.clip(out + cheb_b, -1.0, 1.0)
    ymax = np.full((E, h.shape[1]), -np.inf, f32)
    ymin = np.full((E, h.shape[1]), np.inf, f32)
    np.maximum.at(ymax, batch, h)
    np.minimum.at(ymin, batch, h)
    ynorm = np.zeros((E, h.shape[1]), f32)
    np.add.at(ynorm, batch, h * h)
    ynorm = np.sqrt(ynorm / cnt[:, None])
    y = np.concatenate([ymax - ymin, ynorm], axis=1)
    logits = y @ lin_W + lin_b
    return (1.0 / (1.0 + np.exp(-logits))).squeeze(-1).astype(f32)


def _has_window_structure(members, edge_index, batch):
    E, Mm = members.shape
    if Mm != M:
        return False
    base = members[:, 0]
    if not (members == base[:, None] + np.arange(M, dtype=members.dtype)).all():
        return False
    if not (batch == np.repeat(np.arange(E, dtype=batch.dtype), M)).all():
        return False
    row, col = np.where(~np.eye(M, dtype=bool))
    offs = np.arange(E, dtype=np.int64)[:, None] * M
    ei = np.stack([(offs + row[None, :]).ravel(), (offs + col[None, :]).ravel()])
    return (edge_index == ei).all()


def kernel(pos_set, W_enc, b_enc, gn_gamma, gn_beta, gn_alpha, cheb_W, cheb_b,
           lin_W, lin_b, members, edge_index, batch):
    pos_set = np.asarray(pos_set, np.float32)
    members = np.asarray(members)
    edge_index = np.asarray(edge_index)
    batch = np.asarray(batch)
    if not _has_window_structure(members, edge_index, batch):
        return _np_fallback(
            pos_set, np.asarray(W_enc, np.float32),
            np.asarray(b_enc, np.float32), np.asarray(gn_gamma, np.float32),
            np.asarray(gn_beta, np.float32), np.asarray(gn_alpha, np.float32),
            np.asarray(cheb_W, np.float32), np.asarray(cheb_b, np.float32),
            np.asarray(lin_W, np.float32), np.asarray(lin_b, np.float32),
            members, edge_index, batch)

    N = pos_set.shape[0]
    E = members.shape[0]
    base = members[:, 0].astype(np.int64)
    node_span = (N + N_CORES - 1) // N_CORES                # 6250
    u_pad = ((node_span + M + 511) // 512 + 1) * 512        # 6656 for N=50000
    # quantile split: sort windows by base, give each core an equal count.
    # Cores then own contiguous (narrow) base bands; sorted bases inside a
    # core give gather chunks narrow node ranges -> partial x-write deps
    # (pipelining). Falls back to equal-node split if a band's node span
    # exceeds u_pad.
    order = np.argsort(base, kind="stable")
    ecnt = (E + N_CORES - 1) // N_CORES
    counts = np.array([min(ecnt, E - c * ecnt) for c in range(N_CORES)])
    offs_pre = np.concatenate([[0], np.cumsum(counts)])
    node_lo = np.zeros(N_CORES, np.int64)
    ok = True
    for c in range(N_CORES):
        ids = order[offs_pre[c]:offs_pre[c + 1]]
        if ids.size == 0:
            node_lo[c] = 0
            continue
        node_lo[c] = base[ids[0]]
        if base[ids[-1]] + M - node_lo[c] > u_pad:
            ok = False
            break
    if not ok:
        core_of = np.minimum(base // node_span, N_CORES - 1)
        order = np.argsort(base, kind="stable")
        counts = np.bincount(core_of, minlength=N_CORES)
        offs_pre = np.concatenate([[0], np.cumsum(counts)])
        node_lo = np.arange(N_CORES, dtype=np.int64) * node_span
    e_pad = max(GSZ, int(math.ceil(counts.max() / GSZ)) * GSZ)

    benc_zero = bool(np.all(np.asarray(b_enc) == 0.0))
    ng_ = e_pad // GSZ
    nwb_ = u_pad // 512
    # per-chunk: how many 512-node x-write batches the gather depends on
    # (max over cores, from the actual window bases)
    gdeps = []
    for c in range(ng_):
        mx = 0
        for cc in range(N_CORES):
            ids = order[offs_pre[cc] + c * GSZ:
                        min(offs_pre[cc] + (c + 1) * GSZ, offs_pre[cc + 1])]
            if ids.size:
                mx = max(mx, int((base[ids] - node_lo[cc]).max()))
        gdeps.append(min(nwb_, (mx + M + 511) // 512))
    gdeps = tuple(gdeps)
    key = (u_pad, e_pad, benc_zero, gdeps)
    if key not in _GRAPH_CACHE:
        _GRAPH_CACHE[key] = _build_graph(u_pad, e_pad, benc_zero, gdeps)
    nc = _GRAPH_CACHE[key]

    shared = _fold_weights(W_enc, b_enc, gn_gamma, gn_beta, gn_alpha, cheb_W,
                           cheb_b, lin_W, lin_b)
    nb = u_pad // 512
    ns16 = e_pad // 16

    in_maps = []
    offs = offs_pre
    for c in range(N_CORES):
        lo = int(node_lo[c])
        sl = pos_set[lo:min(lo + u_pad, N)]
        if sl.shape[0] < u_pad:
            sl = np.concatenate(
                [sl, np.zeros((u_pad - sl.shape[0], F), np.float32)], 0)
        # posT[b, p, k, u'] = sl[512b+u', 128k+p]
        posT = np.ascontiguousarray(
            sl.reshape(nb, 512, 4, 128).transpose(0, 3, 2, 1)
            .astype(_bf16_dtype()))
        ids = order[offs[c]:offs[c + 1]]
        loc = (base[ids] - lo).astype(np.int64)
        idx = np.zeros(e_pad, np.int16)
        idx[:loc.size] = loc.astype(np.int16)
        # wrapped layout: element i lives at [i % 16, i // 16], replicated
        # across the eight 16-partition groups
        w16 = idx.reshape(ns16, 16).T           # [16, ns16]
        m = dict(shared)
        m["posT"] = posT
        m["idx"] = np.ascontiguousarray(np.tile(w16, (8, 1)))
        in_maps.append(m)

    import os

    from concourse.bass_utils import run_bass_kernel_spmd

    trace = bool(os.environ.get("CHESHIRE_TRACE"))
    res = run_bass_kernel_spmd(nc, in_maps, core_ids=list(range(N_CORES)),
                               trace=trace)
    global LAST_EXEC_NS, LAST_RESULT
    LAST_EXEC_NS = res.exec_time_ns
    LAST_RESULT = res
    out_full = np.zeros(E, np.float32)
    for c in range(N_CORES):
        ids = order[offs[c]:offs[c + 1]]
        vals = np.asarray(res.results[c]["out"], np.float32).reshape(-1)
        out_full[ids] = vals[:ids.size]
    return out_full

